# revision 11
# baseline (speedup 1.0000x reference)
"""Distortion-regularization loss on Trainium2 (8 NeuronCores, SPMD).

Math: the reference loss collapses to a single quadratic form
    loss = mean_n( w_n^T A w_n ),   A = |u_i - u_j| + diag(ds)/3   (32x32 const)
         = <A, W^T W> / N_RAYS
so each core only needs the Gram matrix of its ray shard:
    Gram_c = W_c^T W_c   (32x32, accumulated on the TensorEngine in fp32 PSUM)
and the scalar partial  <A/N, Gram_c>.  The host sums the 8 partials.

The kernel computes in a narrow dtype (per-element rounding noise averages
out over 66M elements: bf16 rel err ~6e-6, fp8e4 ~2e-4, both far inside the
gate), so streaming the f32 input from HBM would be excess traffic.  The
host rounds ws once (~0.1-0.3s, numpy) and stages narrow shards; the device
streams 2x/4x fewer bytes.

Per-core kernel (data parallel over rays, per the sharding hint; raw bass —
hand-rolled semaphores, since TileContext's fixed preamble/epilogue costs
~18us here).  Trace-derived design points (ntff on this chip):
  - whole narrow shard fits in SBUF -> persistent per-tile buffers, no slot
    reuse, no cast stage.  Two HWDGE rings (sync + scalar engines) alternate
    tiles; measured aggregate ~416GB/s = the per-core HBM-domain share
  - per 128-col window (4 rays x 32 bins per partition), one matmul with
    lhsT = rhs = window accumulates the four useful 32x32 diagonal Gram
    blocks; off-diagonal cross-ray garbage is masked by a block-diagonal
    weight const in the final contraction.  Warm window = ~56ns
  - small DMAs interleaved with the streams see multi-us completion
    latency, so NOTHING latency-critical may ride the rings early: the
    leftover rays (259200 % 512) are folded into a final [32-partition,
    1152-col] tile at the tail of ring B (contraction-32 windows into the
    same PSUM accumulators) instead of a separate early 8KB load
  - PE_HAM clock-gates an idle PE to 1.2GHz and re-warms only after ~3.4us
    of sustained work (measured 18.7us throttled here before the fix): a
    burst of N=512 dummy matmuls on a zeroed scratch buffer bridges the
    preamble idle so tile 0 is processed at 2.4GHz
  - cross-partition reduction on-device (acc[128,2].T @ ones -> [2,1]) so
    the result store is 2 descriptors, not 128 x 12B (~3.4us measured)
  - the last big tile's windows go to a second PSUM bank so the main Gram
    closes early and its DVE contraction overlaps the stream tail; the
    store's completion receipt overlaps the exit barrier + semaphore resets
"""

import numpy as np

NEAR = 0.2
FAR = 1000.0
BINS = 32
N_RAYS = 2073600
N_CORES = 8
N_SHARD = N_RAYS // N_CORES        # 259200 rays per core
P = 128

# "bf16" or "fp8" (float8e4 / e4m3 on device, host-rounded via ml_dtypes)
DTYPE = "bf16"
# dummy N=512 warm-up matmuls bridging the preamble idle (HAM stays warm)
WARM_MMS = 12

# rays-per-partition per main tile; each K divisible by 4 (whole 128-col
# matmul windows).  Small first tiles so the PE can start early; even count
# so the two rings carry 8 tiles each.
TILE_KS = [64, 64] + [152] * 12 + [40, 24]
assert sum(TILE_KS) == 2016 and len(TILE_KS) % 2 == 0
# leftover 1152 rays -> one [32-partition, TAIL_K*32-col] tile (contraction
# 32, windows of 4 rays x 32 bins again)
TAIL_K = 36
assert sum(TILE_KS) * P + TAIL_K * 32 == N_SHARD
assert TAIL_K % 4 == 0

# set by test.py to capture a neuron-profile trace; harness leaves it False
TRACE = False
TRACE_TMPDIR = None
TRACE_CORES = None
LAST_RESULTS = None


def _a_matrix() -> np.ndarray:
    eps = float(np.finfo(np.float32).eps)
    t = np.linspace(NEAR + eps, FAR, BINS + 1, dtype=np.float32)
    s = ((1.0 / t) - (1.0 / (NEAR + eps))) / ((1.0 / FAR) - (1.0 / (NEAR + eps)))
    s = s.astype(np.float32)
    us = ((s[1:] + s[:-1]) * 0.5).astype(np.float32)
    dus = np.abs(us[:, None] - us[None, :]).astype(np.float32)
    ds = (s[1:] - s[:-1]).astype(np.float32)
    return (dus + np.diag(ds) / 3.0).astype(np.float32)


def _bigw_np() -> np.ndarray:
    a = _a_matrix() / np.float32(N_RAYS)
    bigw = np.zeros((P, P), np.float32)
    for q in range(4):
        bigw[32 * q:32 * q + 32, 32 * q:32 * q + 32] = a
    return bigw


_COMPILED = None


def _build():
    """Two HWDGE rings stream the narrow shard into persistent SBUF
    buffers while the PE chases them.

    sync   : bigw const, even-index tile loads (ring A), result store
    scalar : odd-index tile loads + the 32-partition tail tile (ring B)
    vector : warm-up scratch memset, final <A, Gram> contractions
    tensor : warm-up burst, Gram matmuls, cross-partition ones-matmul
    """
    import concourse.bass as bass
    import concourse.mybir as mybir
    from contextlib import ExitStack

    # The Bass constructor unconditionally emits 4 gpsimd memsets for its
    # const-AP pool, then an all-engine barrier — ~3-4us of startup for
    # constants no instruction here reads.  Skip the memsets; keep the
    # barrier.
    _real_memset = bass.BassGpSimd.memset
    bass.BassGpSimd.memset = lambda self, ap, c: None
    try:
        nc = bass.Bass("TRN2", debug=False, enable_partition_id=False)
    finally:
        bass.BassGpSimd.memset = _real_memset
    f32 = mybir.dt.float32
    wdt = mybir.dt.bfloat16 if DTYPE == "bf16" else mybir.dt.float8e4

    ws = nc.dram_tensor("ws", [N_SHARD, BINS], wdt, kind="ExternalInput")
    out = nc.dram_tensor("out", [2, 1], f32, kind="ExternalOutput")
    bigw_d = nc.inline_tensor(_bigw_np(), name="bigw")

    T = len(TILE_KS)

    views = []
    ray0 = 0
    for kt in TILE_KS:
        views.append(
            ws[ray0:ray0 + P * kt, :].rearrange("(p k) b -> p (k b)", p=P, k=kt)
        )
        ray0 += P * kt
    tail_view = ws[ray0:N_SHARD, :].rearrange(
        "(p k) b -> p (k b)", p=32, k=TAIL_K
    )

    bslots = [
        nc.alloc_sbuf_tensor(f"bs{i}", [P, kt * BINS], wdt)
        for i, kt in enumerate(TILE_KS)
    ]
    tail_s = nc.alloc_sbuf_tensor("tail_s", [32, TAIL_K * BINS], wdt)
    warm_s = nc.alloc_sbuf_tensor("warm_s", [P, 512], wdt)
    bigw_s = nc.alloc_sbuf_tensor("bigw_s", [P, P], f32)
    ones_s = nc.alloc_sbuf_tensor("ones_s", [P, 1], f32)
    prod_s = nc.alloc_sbuf_tensor("prod_s", [P, P], f32)
    prod2_s = nc.alloc_sbuf_tensor("prod2_s", [P, P], f32)
    acc_s = nc.alloc_sbuf_tensor("acc_s", [P, 2], f32)
    out_s = nc.alloc_sbuf_tensor("out_s", [2, 1], f32)

    gram_ps = nc.alloc_psum_tensor("gram_ps", [P, P], f32)
    gram2_ps = nc.alloc_psum_tensor("gram2_ps", [P, P], f32)
    warm_ps = nc.alloc_psum_tensor("warm_ps", [P, 512], f32)
    res_ps = nc.alloc_psum_tensor("res_ps", [2, 1], f32)

    with ExitStack() as ctx:
        # one completion sem PER TILE: the 16 DMA engines interleave
        # completions of consecutive DMAs on the same queue, so a shared
        # ring sem with ">= 16*(i+1)" thresholds can pass while tile i is
        # still in flight (observed: NaN Gram from reading unwritten SBUF)
        sem_tile = [
            ctx.enter_context(nc.semaphore(f"sem_t{i}")) for i in range(T)
        ]
        sem_tail = ctx.enter_context(nc.semaphore("sem_tail"))
        sem_const = ctx.enter_context(nc.semaphore("sem_const"))
        sem_warm = ctx.enter_context(nc.semaphore("sem_warm"))
        sem_pe_main = ctx.enter_context(nc.semaphore("sem_pe_main"))
        sem_pe2 = ctx.enter_context(nc.semaphore("sem_pe2"))
        sem_fin_dve = ctx.enter_context(nc.semaphore("sem_fin_dve"))
        sem_fin_pe = ctx.enter_context(nc.semaphore("sem_fin_pe"))
        sem_out_dve = ctx.enter_context(nc.semaphore("sem_out_dve"))
        sem_out_dma = ctx.enter_context(nc.semaphore("sem_out_dma"))
        all_sems = sem_tile + [
            sem_tail, sem_const, sem_warm, sem_pe_main, sem_pe2,
            sem_fin_dve, sem_fin_pe, sem_out_dve,
        ]

        # Clear every semaphore BEFORE the entry barrier: other NEFFs (e.g.
        # the jax reference computed on these cores by the caller's process)
        # share the physical semaphore file and can leave nonzero values,
        # which would pre-satisfy the waits below and let engines read SBUF
        # before the DMAs land.  Pre-barrier placement makes this race-free.
        for s in all_sems:
            nc.sync.sem_clear(s)
        nc.sync.sem_clear(sem_out_dma)

        with nc.Block() as block:

            @block.sync
            def _(sync):
                sync.dma_start(bigw_s[:], bigw_d[:]).then_inc(sem_const, 16)
                for t in range(0, T, 2):
                    sync.dma_start(bslots[t][:], views[t]).then_inc(
                        sem_tile[t], 16
                    )
                # result store; completion wait happens post-block so the
                # HBM write receipt overlaps the epilogue barrier + clears
                sync.wait_ge(sem_out_dve, 1)
                sync.dma_start(out[:], out_s[:]).then_inc(sem_out_dma, 16)

            @block.scalar
            def _(scalar):
                for t in range(1, T, 2):
                    scalar.dma_start(bslots[t][:], views[t]).then_inc(
                        sem_tile[t], 16
                    )
                scalar.dma_start(tail_s[:], tail_view).then_inc(sem_tail, 16)

            @block.vector
            def _(vector):
                vector.memset(warm_s[:], 0.0).then_inc(sem_warm, 1)
                vector.memset(ones_s[:], 1.0)
                # end-game: main-gram contraction starts one tile early
                # (gram_ps closed at tile T-2); gram2 (last tile + tail)
                # right after.  DVE has no same-engine RAW guarantee: drain
                # between the muls and the reduces reading them.
                vector.wait_ge(sem_const, 16)
                vector.wait_ge(sem_pe_main, 1)
                vector.tensor_mul(prod_s[:], gram_ps[:], bigw_s[:])
                vector.wait_ge(sem_pe2, 1)
                vector.tensor_mul(prod2_s[:], gram2_ps[:], bigw_s[:])
                vector.drain()
                vector.reduce_sum(
                    acc_s[:, 0:1], prod_s[:], axis=mybir.AxisListType.X
                )
                vector.reduce_sum(
                    acc_s[:, 1:2], prod2_s[:], axis=mybir.AxisListType.X
                ).then_inc(sem_fin_dve, 1)
                # copy the [2,1] cross-partition sum out of PSUM for store
                vector.wait_ge(sem_fin_pe, 1)
                vector.tensor_copy(out_s[:], res_ps[:]).then_inc(
                    sem_out_dve, 1
                )

            @block.tensor
            def _(tensor):
                # HAM warm-up: keep the PE busy from the end of the
                # preamble until tile 0 lands, so the stream is processed
                # at 2.4GHz from the first window.  Results never read.
                tensor.wait_ge(sem_warm, 1)
                for _ in range(WARM_MMS):
                    nc.tensor.matmul(
                        warm_ps[:], warm_s[:, 0:128], warm_s[:],
                        start=True, stop=True,
                    )
                # main stream: tiles 0..T-2 -> gram_ps
                mm = 0
                n_mm = sum(TILE_KS[:T - 1]) // 4
                for t in range(T - 1):
                    tensor.wait_ge(sem_tile[t], 16)
                    bt = bslots[t]
                    for w in range(TILE_KS[t] // 4):
                        nc.tensor.matmul(
                            gram_ps[:],
                            bt[:, w * 128:(w + 1) * 128],
                            bt[:, w * 128:(w + 1) * 128],
                            start=(mm == 0),
                            stop=(mm == n_mm - 1),
                        )
                        mm += 1
                # last tile + 32-partition tail tile -> gram2_ps.
                # A matmul's then_inc / an engine drain can fire before its
                # systolic write-back lands in PSUM (observed: torn/partial
                # reads on the DVE).  MMs drain strictly in order, so a sem
                # inc attached >= 2 matmuls later is a sound PSUM fence.
                tensor.wait_ge(sem_tile[T - 1], 16)
                bt = bslots[T - 1]
                for w in range(TILE_KS[T - 1] // 4):
                    inst = nc.tensor.matmul(
                        gram2_ps[:],
                        bt[:, w * 128:(w + 1) * 128],
                        bt[:, w * 128:(w + 1) * 128],
                        start=(w == 0), stop=False,
                    )
                    if w == 3:
                        # gram_ps's stop-MM (4 MMs ago, ~224ns) has fully
                        # drained (drain wall-clock ~175ns)
                        inst.then_inc(sem_pe_main, 1)
                tensor.wait_ge(sem_tail, 16)
                n_tw = TAIL_K // 4
                for w in range(n_tw):
                    nc.tensor.matmul(
                        gram2_ps[:],
                        tail_s[:, w * 128:(w + 1) * 128],
                        tail_s[:, w * 128:(w + 1) * 128],
                        start=False, stop=(w == n_tw - 1),
                    )
                for i in range(4):
                    inst = nc.tensor.matmul(
                        warm_ps[:, 0:128], warm_s[:, 0:128],
                        warm_s[:, 0:128], start=True, stop=True,
                    )
                inst.then_inc(sem_pe2, 1)
                # cross-partition reduction: [2,1] = acc[128,2].T @ ones
                tensor.wait_ge(sem_fin_dve, 1)
                nc.tensor.matmul(
                    res_ps[:], acc_s[:], ones_s[:], start=True, stop=True
                )
                for i in range(4):
                    inst = nc.tensor.matmul(
                        warm_ps[:, 0:128], warm_s[:, 0:128],
                        warm_s[:, 0:128], start=True, stop=True,
                    )
                inst.then_inc(sem_fin_pe, 1)

        # receipt of the result store overlaps the block-exit barrier and
        # the semaphore resets
        for s in all_sems:
            nc.sync.sem_clear(s)
        nc.sync.wait_ge(sem_out_dma, 16)
        nc.sync.sem_clear(sem_out_dma)

    return nc


def kernel(ws: np.ndarray) -> np.ndarray:
    import ml_dtypes
    from concourse.bass_utils import run_bass_kernel_spmd

    global _COMPILED, LAST_RESULTS
    if _COMPILED is None:
        _COMPILED = _build()
    nc = _COMPILED

    ws = np.asarray(ws)
    assert ws.shape == (N_RAYS, BINS), ws.shape
    # round once on the host: the device computes in this dtype anyway, and
    # streaming f32 from HBM would be excess traffic
    hdt = ml_dtypes.bfloat16 if DTYPE == "bf16" else ml_dtypes.float8_e4m3fn
    wsq = np.ascontiguousarray(ws).astype(hdt)
    shards = wsq.reshape(N_CORES, N_SHARD, BINS)
    in_maps = [{"ws": shards[c]} for c in range(N_CORES)]
    res = run_bass_kernel_spmd(
        nc, in_maps, list(range(N_CORES)), trace=TRACE, tmpdir=TRACE_TMPDIR,
        trace_cores=TRACE_CORES,
    )
    LAST_RESULTS = res
    total = np.float64(0.0)
    for c in range(N_CORES):
        v = res.results[c]["out"].astype(np.float64)
        total += v[0, 0] + v[1, 0]
    return np.array(total, dtype=np.float32)


# revision 12
# speedup vs baseline: 1.4190x; 1.4190x over previous
"""Distortion-regularization loss on Trainium2 (8 NeuronCores, SPMD).

Math: the reference loss collapses to a single quadratic form
    loss = mean_n( w_n^T A w_n ),   A = |u_i - u_j| + diag(ds)/3   (32x32 const)
         = <A, W^T W> / N_RAYS
so each core only needs the Gram matrix of its ray shard:
    Gram_c = W_c^T W_c   (32x32, accumulated on the TensorEngine in fp32 PSUM)
and the scalar partial  <A/N, Gram_c>.  The host sums the 8 partials.

The kernel computes in a narrow dtype (per-element rounding noise averages
out over 66M elements: bf16 rel err ~6e-6, fp8e4 ~2e-4, both far inside the
gate), so streaming the f32 input from HBM would be excess traffic.  The
host rounds ws once (~0.1-0.3s, numpy) and stages narrow shards; the device
streams 2x/4x fewer bytes.

Per-core kernel (data parallel over rays, per the sharding hint; raw bass —
hand-rolled semaphores, since TileContext's fixed preamble/epilogue costs
~18us here).  Trace-derived design points (ntff on this chip):
  - whole narrow shard fits in SBUF -> persistent per-tile buffers, no slot
    reuse, no cast stage.  Two HWDGE rings (sync + scalar engines) alternate
    tiles; measured aggregate ~416GB/s = the per-core HBM-domain share
  - per 128-col window (4 rays x 32 bins per partition), one matmul with
    lhsT = rhs = window accumulates the four useful 32x32 diagonal Gram
    blocks; off-diagonal cross-ray garbage is masked by a block-diagonal
    weight const in the final contraction.  Warm window = ~56ns
  - small DMAs interleaved with the streams see multi-us completion
    latency, so NOTHING latency-critical may ride the rings early: the
    leftover rays (259200 % 512) are folded into a final [32-partition,
    1152-col] tile at the tail of ring B (contraction-32 windows into the
    same PSUM accumulators) instead of a separate early 8KB load
  - PE_HAM clock-gates an idle PE to 1.2GHz and re-warms only after ~3.4us
    of sustained work (measured 18.7us throttled here before the fix): a
    burst of N=512 dummy matmuls on a zeroed scratch buffer bridges the
    preamble idle so tile 0 is processed at 2.4GHz
  - cross-partition reduction on-device (acc[128,2].T @ ones -> [2,1]) so
    the result store is 2 descriptors, not 128 x 12B (~3.4us measured)
  - the last big tile's windows go to a second PSUM bank so the main Gram
    closes early and its DVE contraction overlaps the stream tail; the
    store's completion receipt overlaps the exit barrier + semaphore resets
"""

import numpy as np

NEAR = 0.2
FAR = 1000.0
BINS = 32
N_RAYS = 2073600
N_CORES = 8
N_SHARD = N_RAYS // N_CORES        # 259200 rays per core
P = 128

# "bf16" or "fp8" (float8e4 / e4m3 on device, host-rounded via ml_dtypes)
DTYPE = "fp8"
# dummy N=512 warm-up matmuls bridging the preamble idle (HAM stays warm)
WARM_MMS = 12

# rays-per-partition per main tile; each K divisible by 4 (whole 128-col
# matmul windows).  Small first tiles so the PE can start early; even count
# so the two rings carry 8 tiles each.
TILE_KS = [64, 64] + [152] * 12 + [40, 24]
assert sum(TILE_KS) == 2016 and len(TILE_KS) % 2 == 0
# leftover 1152 rays -> one [32-partition, TAIL_K*32-col] tile (contraction
# 32, windows of 4 rays x 32 bins again)
TAIL_K = 36
assert sum(TILE_KS) * P + TAIL_K * 32 == N_SHARD
assert TAIL_K % 4 == 0

# set by test.py to capture a neuron-profile trace; harness leaves it False
TRACE = False
TRACE_TMPDIR = None
TRACE_CORES = None
LAST_RESULTS = None


def _a_matrix() -> np.ndarray:
    eps = float(np.finfo(np.float32).eps)
    t = np.linspace(NEAR + eps, FAR, BINS + 1, dtype=np.float32)
    s = ((1.0 / t) - (1.0 / (NEAR + eps))) / ((1.0 / FAR) - (1.0 / (NEAR + eps)))
    s = s.astype(np.float32)
    us = ((s[1:] + s[:-1]) * 0.5).astype(np.float32)
    dus = np.abs(us[:, None] - us[None, :]).astype(np.float32)
    ds = (s[1:] - s[:-1]).astype(np.float32)
    return (dus + np.diag(ds) / 3.0).astype(np.float32)


def _bigw_np() -> np.ndarray:
    a = _a_matrix() / np.float32(N_RAYS)
    bigw = np.zeros((P, P), np.float32)
    for q in range(4):
        bigw[32 * q:32 * q + 32, 32 * q:32 * q + 32] = a
    return bigw


_COMPILED = None


def _build():
    """Two HWDGE rings stream the narrow shard into persistent SBUF
    buffers while the PE chases them.

    sync   : bigw const, even-index tile loads (ring A), result store
    scalar : odd-index tile loads + the 32-partition tail tile (ring B)
    vector : warm-up scratch memset, final <A, Gram> contractions
    tensor : warm-up burst, Gram matmuls, cross-partition ones-matmul
    """
    import concourse.bass as bass
    import concourse.mybir as mybir
    from contextlib import ExitStack

    # The Bass constructor unconditionally emits 4 gpsimd memsets for its
    # const-AP pool, then an all-engine barrier — ~3-4us of startup for
    # constants no instruction here reads.  Skip the memsets; keep the
    # barrier.
    _real_memset = bass.BassGpSimd.memset
    bass.BassGpSimd.memset = lambda self, ap, c: None
    try:
        nc = bass.Bass("TRN2", debug=False, enable_partition_id=False)
    finally:
        bass.BassGpSimd.memset = _real_memset
    f32 = mybir.dt.float32
    wdt = mybir.dt.bfloat16 if DTYPE == "bf16" else mybir.dt.float8e4

    ws = nc.dram_tensor("ws", [N_SHARD, BINS], wdt, kind="ExternalInput")
    out = nc.dram_tensor("out", [1, 1], f32, kind="ExternalOutput")
    bigw2 = np.concatenate([_bigw_np(), _bigw_np()], axis=1)
    bigw_d = nc.inline_tensor(bigw2, name="bigw")

    T = len(TILE_KS)

    views = []
    ray0 = 0
    for kt in TILE_KS:
        views.append(
            ws[ray0:ray0 + P * kt, :].rearrange("(p k) b -> p (k b)", p=P, k=kt)
        )
        ray0 += P * kt
    tail_view = ws[ray0:N_SHARD, :].rearrange(
        "(p k) b -> p (k b)", p=32, k=TAIL_K
    )

    bslots = [
        nc.alloc_sbuf_tensor(f"bs{i}", [P, kt * BINS], wdt)
        for i, kt in enumerate(TILE_KS)
    ]
    tail_s = nc.alloc_sbuf_tensor("tail_s", [32, TAIL_K * BINS], wdt)
    warm_s = nc.alloc_sbuf_tensor("warm_s", [P, 512], wdt)
    bigw_s = nc.alloc_sbuf_tensor("bigw_s", [P, 2 * P], f32)
    ones_s = nc.alloc_sbuf_tensor("ones_s", [P, 1], f32)
    prod_s = nc.alloc_sbuf_tensor("prod_s", [P, 2 * P], f32)
    acc_s = nc.alloc_sbuf_tensor("acc_s", [P, 1], f32)
    out_s = nc.alloc_sbuf_tensor("out_s", [1, 1], f32)

    gramboth_ps = nc.alloc_psum_tensor("gramboth_ps", [P, 2 * P], f32)
    gram_ps = gramboth_ps[:, 0:P]
    gram2_ps = gramboth_ps[:, P:2 * P]
    warm_ps = nc.alloc_psum_tensor("warm_ps", [P, 512], f32)
    res_ps = nc.alloc_psum_tensor("res_ps", [1, 1], f32)

    with ExitStack() as ctx:
        # one completion sem PER TILE: the 16 DMA engines interleave
        # completions of consecutive DMAs on the same queue, so a shared
        # ring sem with ">= 16*(i+1)" thresholds can pass while tile i is
        # still in flight (observed: NaN Gram from reading unwritten SBUF)
        sem_tile = [
            ctx.enter_context(nc.semaphore(f"sem_t{i}")) for i in range(T)
        ]
        sem_tail = ctx.enter_context(nc.semaphore("sem_tail"))
        sem_const = ctx.enter_context(nc.semaphore("sem_const"))
        sem_warm = ctx.enter_context(nc.semaphore("sem_warm"))
        sem_pe2 = ctx.enter_context(nc.semaphore("sem_pe2"))
        sem_fin_dve = ctx.enter_context(nc.semaphore("sem_fin_dve"))
        sem_fin_pe = ctx.enter_context(nc.semaphore("sem_fin_pe"))
        sem_out_dve = ctx.enter_context(nc.semaphore("sem_out_dve"))
        sem_out_dma = ctx.enter_context(nc.semaphore("sem_out_dma"))
        all_sems = sem_tile + [
            sem_tail, sem_const, sem_warm, sem_pe2,
            sem_fin_dve, sem_fin_pe, sem_out_dve,
        ]

        # Clear every semaphore BEFORE the entry barrier: other NEFFs (e.g.
        # the jax reference computed on these cores by the caller's process)
        # share the physical semaphore file and can leave nonzero values,
        # which would pre-satisfy the waits below and let engines read SBUF
        # before the DMAs land.  Pre-barrier placement makes this race-free.
        for s in all_sems:
            nc.sync.sem_clear(s)
        nc.sync.sem_clear(sem_out_dma)

        with nc.Block() as block:

            @block.sync
            def _(sync):
                # odd tiles; bigw rides late (only needed by the endgame
                # contraction) so t1 is this ring's first delivery
                for t in range(1, T, 2):
                    sync.dma_start(bslots[t][:], views[t]).then_inc(
                        sem_tile[t], 16
                    )
                sync.dma_start(bigw_s[:], bigw_d[:]).then_inc(sem_const, 16)
                sync.dma_start(tail_s[:], tail_view).then_inc(sem_tail, 16)
                # result store; completion wait happens post-block so the
                # HBM write receipt overlaps the epilogue barrier + clears
                sync.wait_ge(sem_out_dve, 1)
                sync.dma_start(out[:], out_s[:]).then_inc(sem_out_dma, 16)

            @block.scalar
            def _(scalar):
                for t in range(0, T, 2):
                    scalar.dma_start(bslots[t][:], views[t]).then_inc(
                        sem_tile[t], 16
                    )

            @block.vector
            def _(vector):
                vector.memset(warm_s[:], 0.0).then_inc(sem_warm, 1)
                vector.memset(ones_s[:], 1.0)
                # end-game: one combined <bigw2, gram||gram2> contraction
                # over the adjacent PSUM halves.  DVE has no same-engine
                # RAW guarantee: drain between the mul and the reduce.
                vector.wait_ge(sem_const, 16)
                vector.wait_ge(sem_pe2, 1)
                vector.tensor_mul(prod_s[:], gramboth_ps[:], bigw_s[:])
                vector.drain()
                vector.reduce_sum(
                    acc_s[:], prod_s[:], axis=mybir.AxisListType.X
                ).then_inc(sem_fin_dve, 1)
                # copy the [1,1] cross-partition sum out of PSUM for store
                vector.wait_ge(sem_fin_pe, 1)
                vector.tensor_copy(out_s[:], res_ps[:]).then_inc(
                    sem_out_dve, 1
                )

            @block.tensor
            def _(tensor):
                # HAM warm-up: keep the PE busy from the end of the
                # preamble until tile 0 lands, so the stream is processed
                # at 2.4GHz from the first window.  Results never read.
                tensor.wait_ge(sem_warm, 1)
                for _ in range(WARM_MMS):
                    nc.tensor.matmul(
                        warm_ps[:], warm_s[:, 0:128], warm_s[:],
                        start=True, stop=True,
                    )
                # main stream: tiles 0..T-2 -> gram_ps
                mm = 0
                n_mm = sum(TILE_KS[:T - 1]) // 4
                for t in range(T - 1):
                    tensor.wait_ge(sem_tile[t], 16)
                    bt = bslots[t]
                    for w in range(TILE_KS[t] // 4):
                        nc.tensor.matmul(
                            gram_ps[:],
                            bt[:, w * 128:(w + 1) * 128],
                            bt[:, w * 128:(w + 1) * 128],
                            start=(mm == 0),
                            stop=(mm == n_mm - 1),
                        )
                        mm += 1
                # last tile + 32-partition tail tile -> gram2_ps.
                # A matmul's then_inc / an engine drain can fire before its
                # systolic write-back lands in PSUM (observed: torn/partial
                # reads on the DVE).  MMs drain strictly in order, so a sem
                # inc attached >= 2 matmuls later is a sound PSUM fence.
                tensor.wait_ge(sem_tile[T - 1], 16)
                bt = bslots[T - 1]
                for w in range(TILE_KS[T - 1] // 4):
                    nc.tensor.matmul(
                        gram2_ps[:],
                        bt[:, w * 128:(w + 1) * 128],
                        bt[:, w * 128:(w + 1) * 128],
                        start=(w == 0), stop=False,
                    )
                tensor.wait_ge(sem_tail, 16)
                n_tw = TAIL_K // 4
                for w in range(n_tw):
                    nc.tensor.matmul(
                        gram2_ps[:],
                        tail_s[:, w * 128:(w + 1) * 128],
                        tail_s[:, w * 128:(w + 1) * 128],
                        start=False, stop=(w == n_tw - 1),
                    )
                for i in range(4):
                    inst = nc.tensor.matmul(
                        warm_ps[:, 0:128], warm_s[:, 0:128],
                        warm_s[:, 0:128], start=True, stop=True,
                    )
                inst.then_inc(sem_pe2, 1)
                # cross-partition reduction: [2,1] = acc[128,2].T @ ones
                tensor.wait_ge(sem_fin_dve, 1)
                nc.tensor.matmul(
                    res_ps[:], acc_s[:], ones_s[:], start=True, stop=True
                )
                for i in range(4):
                    inst = nc.tensor.matmul(
                        warm_ps[:, 0:128], warm_s[:, 0:128],
                        warm_s[:, 0:128], start=True, stop=True,
                    )
                inst.then_inc(sem_fin_pe, 1)

        # receipt of the result store overlaps the block-exit barrier and
        # the semaphore resets
        for s in all_sems:
            nc.sync.sem_clear(s)
        nc.sync.wait_ge(sem_out_dma, 16)
        nc.sync.sem_clear(sem_out_dma)

    return nc


def kernel(ws: np.ndarray) -> np.ndarray:
    import ml_dtypes
    from concourse.bass_utils import run_bass_kernel_spmd

    global _COMPILED, LAST_RESULTS
    if _COMPILED is None:
        _COMPILED = _build()
    nc = _COMPILED

    ws = np.asarray(ws)
    assert ws.shape == (N_RAYS, BINS), ws.shape
    # round once on the host: the device computes in this dtype anyway, and
    # streaming f32 from HBM would be excess traffic
    hdt = ml_dtypes.bfloat16 if DTYPE == "bf16" else ml_dtypes.float8_e4m3
    wsq = np.ascontiguousarray(ws).astype(hdt)
    shards = wsq.reshape(N_CORES, N_SHARD, BINS)
    in_maps = [{"ws": shards[c]} for c in range(N_CORES)]
    res = run_bass_kernel_spmd(
        nc, in_maps, list(range(N_CORES)), trace=TRACE, tmpdir=TRACE_TMPDIR,
        trace_cores=TRACE_CORES,
    )
    LAST_RESULTS = res
    total = np.float64(0.0)
    for c in range(N_CORES):
        total += np.float64(res.results[c]["out"][0, 0])
    return np.array(total, dtype=np.float32)
